# revision 41
# baseline (speedup 1.0000x reference)
"""3-layer GATv2 (PyG GATv2Conv semantics) on 8 Trainium2 NeuronCores.

Distribution: nodes sharded 12500/core; edges (incl. self-loops) partitioned
by dst core, grouped into 128-dst-node blocks. Per layer:
  phase A: [xl|xr] = h @ [Wl|Wr] for local nodes (PE), rows bf16,
           AllGather -> every core reads all rows from the shared buffer.
  phase B: per superblock of blocks, batched dma_gather of xl[src] only
           (int16 idx; global src rows via src%4 class split). xr[dst] is
           NOT gathered: per slot, the one-hot S [lane, dst] (DVE iota
           compare) is PE-transposed to S_T and a PE matmul S_T^T @ xr_blk
           materializes xr per edge in PSUM. z = xl[src]+xr[dst] (DVE),
           lrelu on Scalar (alpha=0.2), score = reduce(z*a) (DVE),
           w = exp(score) (Scalar), per-block indicator matmul S.T @
           [w*xg | w] accumulates weighted sums + denominators in PSUM.
           Superblock-batched divide/bias/ELU; per-block PE transpose emits
           h_T for the next layer. Layer 3: divide, head-mean, f32 output.
"""
import sys
sys.path.insert(0, "/opt/trn_rl_repo")
import numpy as np
import ml_dtypes

N = 100000
E = 800000
NCORES = 8
SHARD = N // NCORES        # 12500
P = 128
NBLK = (SHARD + P - 1) // P  # 98
SB = 4                      # node blocks per superblock
FIN = 64
H = 4
C1, C3 = 16, 32
F1 = H * C1                # 64
F3 = H * C3                # 128
NEG_SLOPE = 0.2

BF16 = ml_dtypes.bfloat16

_cache = {}


class Meta:
    pass


def _preprocess(edge_index):
    """Sort edges by dst; per (core, block) split by src parity; pad each run
    to x128 (uniform across cores). Group order per superblock:
    class-major, block-minor. Returns per-core idx arrays + graph meta."""
    src = np.concatenate([edge_index[0], np.arange(N, dtype=np.int32)])
    dst = np.concatenate([edge_index[1], np.arange(N, dtype=np.int32)])
    order = np.argsort(dst, kind="stable")
    src_s = src[order].astype(np.int64)
    dst_s = dst[order].astype(np.int64)

    core = dst_s // SHARD
    blk = (dst_s - core * SHARD) // P
    key = core * NBLK + blk
    cnt = np.bincount(key, minlength=NCORES * NBLK).reshape(NCORES, NBLK)
    starts = np.concatenate([[0], np.cumsum(cnt.reshape(-1))])

    # self-edges (src==dst) are handled on-device from local rows; count
    # multiplicity per node and exclude them from the gather classes.
    loop_m = src_s == dst_s
    mult = np.bincount(dst_s[loop_m], minlength=N).astype(np.float32)

    # src classes by direct 32768-row ranges (int16-exact, stride-1 tables);
    # runs kept per (core, sb, class) densely packed (slots may span blocks)
    NCLS = 4
    NSB = (NBLK + SB - 1) // SB
    gc = np.zeros((NCORES, NSB, NCLS), np.int64)
    runs = {}
    for c in range(NCORES):
        for s in range(NSB):
            blo, bhi = s * SB, min(NBLK, (s + 1) * SB)
            i0, i1 = c * NBLK + blo, c * NBLK + bhi
            e0, e1 = starts[i0], starts[i1]
            nl = ~loop_m[e0:e1]
            sr, dr = src_s[e0:e1][nl], dst_s[e0:e1][nl]
            cls = sr >> 15
            for r in range(NCLS):
                sel = cls == r
                runs[(c, s, r)] = (sr[sel], dr[sel])
                gc[c, s, r] = sel.sum()
    Gc = np.maximum(1, -(-gc.max(axis=0) // P))   # [NSB, NCLS] groups per run
    mult_pad = np.zeros((NCORES, NBLK * P), np.float32)
    mult_pad[:, :SHARD] = mult.reshape(NCORES, SHARD)
    multc = mult_pad.reshape(NCORES, NBLK, P).transpose(0, 2, 1).astype(BF16).copy()

    m = Meta()
    m.NCLS = NCLS
    m.NSB = NSB
    m.sb_blocks = [list(range(s * SB, min(NBLK, (s + 1) * SB))) for s in range(NSB)]
    m.sb_cls_off = []  # per sb: [o0..o4] class group (slot) offsets
    m.sb_g = []        # per sb: total slots
    for s in range(NSB):
        offs = [0]
        for r in range(NCLS):
            offs.append(offs[-1] + int(Gc[s, r]))
        m.sb_cls_off.append(offs)
        m.sb_g.append(offs[-1])
    m.SBGmax = max(m.sb_g)
    m.Gtot = sum(m.sb_g)

    # pairs (slot, block) per sb: union over cores of blocks intersecting
    # each slot's dense lane range
    m.sb_pairs = []      # per sb: ordered list of (slot, a)
    m.blk_pairs = []     # per sb: {a: [pair ids]}
    m.slot_pairs = []    # per sb: {slot: [pair ids]}
    for s in range(NSB):
        blo = s * SB
        pairset = set()
        for c in range(NCORES):
            for r in range(NCLS):
                offr = m.sb_cls_off[s][r]
                sr, dr = runs[(c, s, r)]
                blk = (dr - c * SHARD) // P - blo
                for k in range(int(Gc[s, r])):
                    seg = blk[k * P:(k + 1) * P]
                    for a in np.unique(seg):
                        pairset.add((offr + k, int(a)))
        pl = sorted(pairset)
        m.sb_pairs.append(pl)
        bp = {}
        sp = {}
        for pid, (sl, a) in enumerate(pl):
            bp.setdefault(a, []).append(pid)
            sp.setdefault(sl, []).append(pid)
        m.blk_pairs.append(bp)
        m.slot_pairs.append(sp)
    m.sb_p = [len(pl) for pl in m.sb_pairs]
    m.NPmax = max(m.sb_p)
    m.Ptot = sum(m.sb_p)
    sb_poff = np.concatenate([[0], np.cumsum(m.sb_p)])
    m.sb_poff = [int(v) for v in sb_poff]

    # per-core flat src idx arrays in (sb, slot, lane) order; rel per PAIR.
    # pads = -1: the Q7 trims trailing negatives, so padding costs no descs.
    src_idx = np.full((NCORES, m.Gtot * P), -1, np.int16)
    rel = np.full((NCORES, m.Ptot * P), -1.0, np.float32)
    sb_goff = np.concatenate([[0], np.cumsum(m.sb_g)])
    m.sb_goff = [int(v) for v in sb_goff]
    for c in range(NCORES):
        for s in range(NSB):
            blo = s * SB
            base = sb_goff[s] * P
            pbase = m.sb_poff[s] * P
            pl = m.sb_pairs[s]
            pair_id = {pa: i for i, pa in enumerate(pl)}
            for r in range(NCLS):
                offr = m.sb_cls_off[s][r]
                sr, dr = runs[(c, s, r)]
                o = base + offr * P
                src_idx[c, o:o + len(sr)] = sr - (r << 15)
                # pad with idx 0 up to the cross-core max count (kept valid so
                # num_idxs_reg == nonneg-count holds on every core); beyond
                # that stays -1 and the Q7 trims it at zero descriptor cost
                maxcr = int(gc[:, s, r].max())
                src_idx[c, o + len(sr):o + maxcr] = 0
                drel = dr - c * SHARD
                blk = drel // P - blo
                for i in range(len(sr)):
                    sl = offr + i // P
                    pid = pair_id[(sl, int(blk[i]))]
                    rel[c, pbase + pid * P + i % P] = drel[i] - (blk[i] + blo) * P
    m.Gc = Gc
    m.gc_max = gc.max(axis=0)   # [NSB, NCLS] per-call valid count (uniform)
    m.multc = multc

    def wrap(a):
        w = a.reshape(NCORES, m.Gtot * P // 16, 16).transpose(0, 2, 1)
        return np.tile(w, (1, 8, 1)).copy()

    m.src_w = wrap(src_idx)
    m.rel_pm = rel.reshape(NCORES, m.Ptot, P).transpose(0, 2, 1).astype(BF16).copy()
    return m


def _build_program(m):
    import concourse.bass as bass
    import concourse.bacc as bacc
    import concourse.tile as tile
    from concourse import mybir, library_config

    bf16, f32, i16 = mybir.dt.bfloat16, mybir.dt.float32, mybir.dt.int16
    AF = mybir.ActivationFunctionType
    OP = mybir.AluOpType
    X = mybir.AxisListType.X

    nc = bacc.Bacc("TRN2", target_bir_lowering=False)

    WI = m.Gtot * P // 16
    xT_d = nc.dram_tensor("xT", [FIN, SHARD], bf16, kind="ExternalInput")
    multb_d = nc.dram_tensor("multb", [P, NBLK], bf16, kind="ExternalInput")
    srcw_d = nc.dram_tensor("srcw", [P, WI], i16, kind="ExternalInput")
    dstrel_d = nc.dram_tensor("dstrel", [P, m.Ptot], bf16, kind="ExternalInput")
    iota_d = nc.dram_tensor("iota", [P, m.NPmax * P], bf16, kind="ExternalInput")
    W_d = [nc.dram_tensor(f"W{l}", [FIN, 2 * (F1 if l < 3 else F3)], bf16, kind="ExternalInput") for l in (1, 2, 3)]
    arep_d = [nc.dram_tensor(f"arep{l}", [P, m.SBGmax * (F1 if l < 3 else F3)], bf16, kind="ExternalInput") for l in (1, 2, 3)]
    brep_d = [nc.dram_tensor("brep1", [P, SB * F1], bf16, kind="ExternalInput"),
              nc.dram_tensor("brep2", [P, SB * F1], bf16, kind="ExternalInput"),
              nc.dram_tensor("brep3", [P, 2 * C3], f32, kind="ExternalInput")]
    ident_d = nc.dram_tensor("ident", [P, P], bf16, kind="ExternalInput")
    out_d = nc.dram_tensor("out_shard", [SHARD, C3], f32, kind="ExternalOutput")

    def internal(name, shape, dt, shared=False):
        return nc.dram_tensor(name, shape, dt, kind="Internal",
                              addr_space="Shared" if shared else "Local")

    hT_next = [internal(f"hT{l}", [F1, SHARD], bf16) for l in (1, 2)]
    xlr_sh = [internal(f"xlrsh{l}", [SHARD, 2 * (F1 if l < 3 else F3)], bf16) for l in (1, 2, 3)]
    xlr_rows_cc = [internal(f"xlrrowscc{l}", [N, 2 * (F1 if l < 3 else F3)], bf16, shared=True) for l in (1, 2, 3)]

    RG = [list(range(NCORES))]

    with tile.TileContext(nc) as tc:
        nc.gpsimd.load_library(library_config.mlp)
        with tc.tile_pool(name="const", bufs=1) as cpool, \
             tc.tile_pool(name="work", bufs=2) as wpool, \
             tc.tile_pool(name="mmA", bufs=3) as apool, \
             tc.tile_pool(name="tail", bufs=2) as tpool, \
             tc.tile_pool(name="psAT", bufs=2, space="PSUM") as ppAT, \
             tc.tile_pool(name="psB", bufs=2, space="PSUM") as ppB, \
             tc.tile_pool(name="psST", bufs=2, space="PSUM") as ppST, \
             tc.tile_pool(name="psXR", bufs=2, space="PSUM") as ppXR:

            srcw = cpool.tile([P, WI], i16)
            dstrel = cpool.tile([P, m.Ptot], bf16)
            iota = cpool.tile([P, m.NPmax * P], bf16)
            ident = cpool.tile([P, P], bf16)
            multb = cpool.tile([P, NBLK], bf16)
            for t, d in [(srcw, srcw_d), (dstrel, dstrel_d),
                         (iota, iota_d), (ident, ident_d), (multb, multb_d)]:
                nc.sync.dma_start(t[:], d[:])
            W_sb, arep_sb, brep_sb = [], [], []
            for li in range(3):
                Fl = F1 if li < 2 else F3
                w = cpool.tile([FIN, 2 * Fl], bf16, tag=f"W{li}")
                nc.sync.dma_start(w[:], W_d[li][:])
                W_sb.append(w)
                a = cpool.tile([P, m.SBGmax * Fl], bf16, tag=f"arep{li}")
                nc.sync.dma_start(a[:], arep_d[li][:])
                arep_sb.append(a)
                b = cpool.tile([P, SB * F1 if li < 2 else 2 * C3],
                               bf16 if li < 2 else f32, tag=f"brep{li}")
                nc.sync.dma_start(b[:], brep_d[li][:])
                brep_sb.append(b)
            # persistent ping-pong gather buffers, memset once so pad lanes
            # (skipped by the Q7's trailing-negative trim) stay finite
            xg0 = cpool.tile([P, m.SBGmax, F3], bf16, tag="xg0")
            xg1 = cpool.tile([P, m.SBGmax, F3], bf16, tag="xg1")
            nc.vector.memset(xg0[:], 0.0)
            nc.vector.memset(xg1[:], 0.0)
            xg_pp = [xg0, xg1]
            # resident xT for phase A
            xTr = cpool.tile([FIN, SHARD], bf16, tag="xTr")
            nc.sync.dma_start(xTr[:], xT_d[:])

            for li in range(3):
                l3 = (li == 2)
                Fl = F3 if l3 else F1
                Cl = C3 if l3 else C1
                FE = 2 * Fl            # row width of xlr tensors

                # ---- phase A (layer 1 only; later layers fused in B tails) ----
                if li == 0:
                    for t in range(NBLK):
                        n0 = t * P
                        mm = min(P, SHARD - n0)
                        psA = ppAT.tile([P, 2 * F3], f32, tag="psAT", space="PSUM")
                        nc.tensor.matmul(psA[:mm, :FE], lhsT=xTr[:, n0:n0 + mm],
                                         rhs=W_sb[li][:], start=True, stop=True)
                        xlr = apool.tile([P, 2 * F3], bf16, tag="xlr")
                        nc.scalar.copy(xlr[:mm, :FE], psA[:mm, :FE])
                        nc.sync.dma_start(xlr_sh[li][n0:n0 + mm, :], xlr[:mm, :FE])

                nc.gpsimd.collective_compute(
                    "AllGather", mybir.AluOpType.bypass, replica_groups=RG,
                    ins=[xlr_sh[li][:]], outs=[xlr_rows_cc[li][:]])

                # gather source views: 4 direct-range classes (idx = src - r*32768)
                if not l3:
                    src_tabs = [xlr_rows_cc[li][r * 32768:, :] for r in range(m.NCLS)]
                    GELEM, GSTEP = FE, FE
                else:
                    src_tabs = [xlr_rows_cc[li][r * 32768:, :F3] for r in range(m.NCLS)]
                    GELEM, GSTEP = F3, FE

                # ---- phase B ----
                for s in range(m.NSB):
                    SG = m.sb_g[s]
                    go = m.sb_goff[s]
                    po = m.sb_poff[s]
                    NP = m.sb_p[s]
                    pairs = m.sb_pairs[s]
                    spairs = m.slot_pairs[s]
                    wo = go * P // 16
                    blocks = m.sb_blocks[s]
                    nblk = len(blocks)
                    xg = xg_pp[s % 2]
                    offs = m.sb_cls_off[s]
                    for r in range(m.NCLS):
                        nr = (offs[r + 1] - offs[r]) * P
                        nvalid = int(m.gc_max[s, r])
                        if nr == 0 or nvalid == 0:
                            continue
                        nc.gpsimd.dma_gather(
                            out_ap=xg[:, offs[r]:offs[r + 1], :GELEM], in_ap=src_tabs[r],
                            idxs_ap=srcw[:, wo + offs[r] * 8:wo + offs[r + 1] * 8],
                            num_idxs=nr, num_idxs_reg=nvalid, elem_size=GELEM,
                            elem_step=GSTEP, single_packet=False)

                    # local [xl|xr] rows per block: [P, SB, FE]
                    xrb = wpool.tile([P, SB, 2 * F3], bf16, tag="xrb")
                    if blocks[-1] * P + P > SHARD:
                        nc.vector.memset(xrb[:], 0.0)
                    for a, b in enumerate(blocks):
                        n0 = b * P
                        mm = min(P, SHARD - n0)
                        nc.sync.dma_start(xrb[:mm, a, :FE],
                                          xlr_sh[li][n0:n0 + mm, :])

                    # one-hot S [lane, dst-in-block] per PAIR (slot, block)
                    S = wpool.tile([P, m.NPmax, P], bf16, tag="S")
                    nc.vector.tensor_tensor(
                        out=S[:, :NP, :],
                        in0=iota[:, :NP * P].rearrange("p (g n) -> p g n", g=NP),
                        in1=dstrel[:, po:po + NP].to_broadcast([P, NP, P]),
                        op=OP.is_equal)

                    # z = xl[src] + xr[dst]; xr via PE: S_T = transpose(S),
                    # then psum_xr[slot] += S_T^T @ xr_blk over the slot's pairs
                    zl = wpool.tile([P, m.SBGmax, F3 if l3 else F1], bf16, tag="zl")
                    psxr_t = {}
                    nqp = (NP + 3) // 4
                    for q in range(nqp):
                        w4 = min(4, NP - 4 * q)
                        psST = ppST.tile([P, 4, P], bf16, tag="psST", space="PSUM")
                        for j in range(w4):
                            nc.tensor.transpose(psST[:, j, :], S[:, 4 * q + j, :], ident[:])
                        STt = wpool.tile([P, 4, P], bf16, tag="STt")
                        if q % 2 == 0:
                            nc.scalar.copy(STt[:, :w4, :], psST[:, :w4, :])
                        else:
                            nc.vector.tensor_copy(STt[:, :w4, :], psST[:, :w4, :])
                        for j in range(w4):
                            pid = 4 * q + j
                            sl, a = pairs[pid]
                            sq = sl // 4
                            if sq not in psxr_t:
                                t_xr = ppXR.tile([P, 4, F3], f32, tag="psXR", space="PSUM")
                                psxr_t[sq] = t_xr
                            plist = spairs[sl]
                            nc.tensor.matmul(psxr_t[sq][:, sl % 4, :Fl],
                                             lhsT=STt[:, j, :],
                                             rhs=xrb[:, a, Fl:FE],
                                             start=(pid == plist[0]),
                                             stop=(pid == plist[-1]))
                            if pid == NP - 1 or pairs[pid + 1][0] // 4 > sq:
                                w4s = min(4, SG - sq * 4)
                                nc.vector.tensor_tensor(
                                    out=zl[:, sq * 4:sq * 4 + w4s, :Fl],
                                    in0=xg[:, sq * 4:sq * 4 + w4s, :Fl],
                                    in1=psxr_t[sq][:, :w4s, :Fl], op=OP.add)
                                del psxr_t[sq]

                    # lrelu = max(0.2 z, z) on DVE, then *att, reduce -> score
                    nc.vector.scalar_tensor_tensor(
                        out=zl[:, :SG, :Fl], in0=zl[:, :SG, :Fl], scalar=NEG_SLOPE,
                        in1=zl[:, :SG, :Fl], op0=OP.mult, op1=OP.max)
                    nc.vector.tensor_tensor(
                        out=zl[:, :SG, :Fl], in0=zl[:, :SG, :Fl],
                        in1=arep_sb[li][:, :SG * Fl].rearrange("p (g f) -> p g f", g=SG),
                        op=OP.mult)
                    score = wpool.tile([P, m.SBGmax * H], f32, tag="score")
                    nc.vector.tensor_reduce(
                        out=score[:, :SG * H],
                        in_=zl[:, :SG, :Fl].rearrange("p g (h c) -> p g h c", h=H),
                        axis=X, op=OP.add)
                    wlhs = wpool.tile([P, m.SBGmax, F3 + H], bf16, tag="wlhs")
                    nc.scalar.activation(
                        wlhs[:, :SG, :Fl].rearrange("p g (h c) -> p g h c", h=H),
                        score[:, :SG * H].rearrange("p (g h) -> p g h", g=SG).to_broadcast([P, SG, H, Cl]),
                        AF.Exp)
                    nc.scalar.activation(
                        wlhs[:, :SG, Fl:Fl + H],
                        score[:, :SG * H].rearrange("p (g h) -> p g h", g=SG),
                        AF.Exp)
                    nc.vector.tensor_tensor(out=wlhs[:, :SG, :Fl],
                                            in0=xg[:, :SG, :Fl],
                                            in1=wlhs[:, :SG, :Fl], op=OP.mult)

                    # self-edge contribution from local rows (no gather):
                    # zs = xl[d]+xr[d]; wx = mult * [exp(a.lrelu(zs)) * xl | exp]
                    zs = tpool.tile([P, SB, F3], bf16, tag="zs")
                    nc.vector.tensor_tensor(out=zs[:, :nblk, :Fl],
                                            in0=xrb[:, :nblk, :Fl],
                                            in1=xrb[:, :nblk, Fl:FE], op=OP.add)
                    nc.vector.scalar_tensor_tensor(
                        out=zs[:, :nblk, :Fl], in0=zs[:, :nblk, :Fl],
                        scalar=NEG_SLOPE, in1=zs[:, :nblk, :Fl],
                        op0=OP.mult, op1=OP.max)
                    nc.vector.tensor_tensor(
                        out=zs[:, :nblk, :Fl], in0=zs[:, :nblk, :Fl],
                        in1=arep_sb[li][:, :nblk * Fl].rearrange("p (a f) -> p a f", a=nblk),
                        op=OP.mult)
                    ssc = tpool.tile([P, SB * H], f32, tag="ssc")
                    nc.vector.tensor_reduce(
                        out=ssc[:, :nblk * H],
                        in_=zs[:, :nblk, :Fl].rearrange("p a (h c) -> p a h c", h=H),
                        axis=X, op=OP.add)
                    wx = tpool.tile([P, SB, F3 + H], bf16, tag="wx")
                    nc.scalar.activation(
                        wx[:, :nblk, :Fl].rearrange("p a (h c) -> p a h c", h=H),
                        ssc[:, :nblk * H].rearrange("p (a h) -> p a h", a=nblk).to_broadcast([P, nblk, H, Cl]),
                        AF.Exp)
                    nc.scalar.activation(
                        wx[:, :nblk, Fl:Fl + H],
                        ssc[:, :nblk * H].rearrange("p (a h) -> p a h", a=nblk),
                        AF.Exp)
                    nc.vector.tensor_tensor(out=wx[:, :nblk, :Fl],
                                            in0=xrb[:, :nblk, :Fl],
                                            in1=wx[:, :nblk, :Fl], op=OP.mult)
                    b0 = blocks[0]
                    nc.vector.tensor_tensor(
                        out=wx[:, :nblk, :Fl + H], in0=wx[:, :nblk, :Fl + H],
                        in1=multb[:, b0:b0 + nblk, None].to_broadcast([P, nblk, Fl + H]),
                        op=OP.mult)

                    # aggregation: per block, accumulate S.T @ [w*xl | w]
                    bpairs = m.blk_pairs[s]
                    if not l3:
                        psB = ppB.tile([P, SB, F1 + H], f32, tag="psB", space="PSUM")
                        for a, b in enumerate(blocks):
                            plist = bpairs[a]
                            for i, pid in enumerate(plist):
                                nc.tensor.matmul(psB[:, a, :Fl + H], lhsT=S[:, pid, :],
                                                 rhs=wlhs[:, pairs[pid][0], :Fl + H],
                                                 start=(i == 0), stop=(i == len(plist) - 1))
                        nc.vector.tensor_tensor(
                            out=psB[:, :nblk, :Fl + H], in0=psB[:, :nblk, :Fl + H],
                            in1=wx[:, :nblk, :Fl + H], op=OP.add)
                        # superblock-batched tail
                        rec = tpool.tile([P, SB, H], f32, tag="rec")
                        nc.vector.reciprocal(rec[:, :nblk, :], psB[:, :nblk, Fl:Fl + H])
                        hb = tpool.tile([P, SB, F1], bf16, tag="hb")
                        nc.vector.tensor_tensor(
                            out=hb[:, :nblk, :].rearrange("p a (h c) -> p a h c", h=H),
                            in0=psB[:, :nblk, :Fl].rearrange("p a (h c) -> p a h c", h=H),
                            in1=rec[:, :nblk, :, None].to_broadcast([P, nblk, H, Cl]),
                            op=OP.mult)
                        nc.vector.tensor_tensor(
                            out=hb[:, :nblk, :],
                            in0=hb[:, :nblk, :],
                            in1=brep_sb[li][:, :nblk * F1].rearrange("p (a f) -> p a f", a=nblk),
                            op=OP.add)
                        rp = tpool.tile([P, SB, F1], bf16, tag="rp")
                        nc.scalar.activation(rp[:, :nblk, :], hb[:, :nblk, :], AF.Relu)
                        xm = tpool.tile([P, SB, F1], bf16, tag="xm")
                        nc.vector.tensor_scalar_min(out=xm[:, :nblk, :],
                                                    in0=hb[:, :nblk, :], scalar1=0.0)
                        ex = tpool.tile([P, SB, F1], f32, tag="ex")
                        nc.scalar.activation(ex[:, :nblk, :], xm[:, :nblk, :], AF.Exp)
                        ho = tpool.tile([P, SB, F1], bf16, tag="ho")
                        nc.vector.scalar_tensor_tensor(
                            out=ho[:, :nblk, :], in0=ex[:, :nblk, :], scalar=-1.0,
                            in1=rp[:, :nblk, :], op0=OP.add, op1=OP.add)
                        for a, b in enumerate(blocks):
                            n0 = b * P
                            mm = min(P, SHARD - n0)
                            psT = ppAT.tile([F1, P], bf16, tag="psAT", space="PSUM")
                            nc.tensor.transpose(psT[:, :mm], ho[:mm, a, :], ident[:mm, :mm])
                            hTn = tpool.tile([F1, P], bf16, tag="hTn")
                            nc.scalar.copy(hTn[:, :mm], psT[:, :mm])
                            # fused phase A of the next layer
                            FEn = 2 * (F1 if li == 0 else F3)
                            psA = ppAT.tile([P, 2 * F3], f32, tag="psAT", space="PSUM")
                            nc.tensor.matmul(psA[:mm, :FEn], lhsT=hTn[:, :mm],
                                             rhs=W_sb[li + 1][:], start=True, stop=True)
                            xlr = apool.tile([P, 2 * F3], bf16, tag="xlr")
                            nc.scalar.copy(xlr[:mm, :FEn], psA[:mm, :FEn])
                            nc.sync.dma_start(xlr_sh[li + 1][n0:n0 + mm, :], xlr[:mm, :FEn])
                    else:
                        for half in range((nblk + 1) // 2):
                            hb0 = 2 * half
                            hblocks = blocks[hb0:hb0 + 2]
                            nh = len(hblocks)
                            psB3 = ppB.tile([P, 2, F3 + H], f32, tag="psB", space="PSUM")
                            for a, b in enumerate(hblocks):
                                plist = bpairs[hb0 + a]
                                for i, pid in enumerate(plist):
                                    nc.tensor.matmul(psB3[:, a, :F3 + H], lhsT=S[:, pid, :],
                                                     rhs=wlhs[:, pairs[pid][0], :F3 + H],
                                                     start=(i == 0), stop=(i == len(plist) - 1))
                            nc.vector.tensor_tensor(
                                out=psB3[:, :nh, :F3 + H], in0=psB3[:, :nh, :F3 + H],
                                in1=wx[:, hb0:hb0 + nh, :F3 + H], op=OP.add)
                            rec = tpool.tile([P, 2, H], f32, tag="rec3")
                            nc.vector.reciprocal(rec[:, :nh, :], psB3[:, :nh, F3:F3 + H])
                            o3 = tpool.tile([P, 2, F3], f32, tag="o3")
                            nc.vector.tensor_tensor(
                                out=o3[:, :nh, :].rearrange("p a (h c) -> p a h c", h=H),
                                in0=psB3[:, :nh, :F3].rearrange("p a (h c) -> p a h c", h=H),
                                in1=rec[:, :nh, :, None].to_broadcast([P, nh, H, Cl]),
                                op=OP.mult)
                            m01 = tpool.tile([P, 2, C3], f32, tag="m01")
                            nc.vector.tensor_tensor(out=m01[:, :nh, :], in0=o3[:, :nh, 0:C3],
                                                    in1=o3[:, :nh, C3:2 * C3], op=OP.add)
                            m23 = tpool.tile([P, 2, C3], f32, tag="m23")
                            nc.vector.tensor_tensor(out=m23[:, :nh, :], in0=o3[:, :nh, 2 * C3:3 * C3],
                                                    in1=o3[:, :nh, 3 * C3:4 * C3], op=OP.add)
                            ms = tpool.tile([P, 2, C3], f32, tag="ms")
                            nc.vector.tensor_tensor(out=ms[:, :nh, :], in0=m01[:, :nh, :],
                                                    in1=m23[:, :nh, :], op=OP.add)
                            of = tpool.tile([P, 2, C3], f32, tag="of")
                            nc.vector.scalar_tensor_tensor(
                                out=of[:, :nh, :], in0=ms[:, :nh, :], scalar=0.25,
                                in1=brep_sb[2][:, :nh * C3].rearrange("p (a c) -> p a c", a=nh),
                                op0=OP.mult, op1=OP.add)
                            for a, b in enumerate(hblocks):
                                n0 = b * P
                                mm = min(P, SHARD - n0)
                                nc.sync.dma_start(out_d[n0:n0 + mm, :], of[:mm, a, :])

    nc.compile()
    return nc


def _prep_inputs(x, edge_index, Ws, atts):
    m = _preprocess(edge_index)
    ident = np.eye(P, dtype=np.float32).astype(BF16)
    iota = np.broadcast_to(np.arange(P, dtype=np.float32), (P, m.NPmax, P)) \
        .reshape(P, m.NPmax * P).astype(BF16).copy()
    common = {"ident": ident, "iota": iota}
    for li, ((Wl, Wr), a) in enumerate(zip(Ws, atts)):
        Fl = Wl.shape[1]
        common[f"W{li + 1}"] = np.concatenate([Wl, Wr], axis=1).astype(BF16)
        a_flat = np.asarray(a).reshape(Fl).astype(np.float32)
        common[f"arep{li + 1}"] = np.broadcast_to(a_flat, (P, m.SBGmax, Fl)) \
            .reshape(P, m.SBGmax * Fl).astype(BF16).copy()
    in_maps = []
    for c in range(NCORES):
        d = dict(common)
        d["xT"] = x[c * SHARD:(c + 1) * SHARD].T.astype(BF16).copy()
        d["srcw"] = m.src_w[c]
        d["dstrel"] = m.rel_pm[c]
        d["multb"] = m.multc[c]
        in_maps.append(d)
    return in_maps, m


def kernel(x, edge_index, W1l, W1r, a1, b1, W2l, W2r, a2, b2, W3l, W3r, a3, b3,
           _trace=False, _tmpdir=None):
    from concourse.bass_utils import run_bass_kernel_spmd

    x = np.asarray(x, dtype=np.float32)
    edge_index = np.asarray(edge_index, dtype=np.int32)
    in_maps, m = _prep_inputs(
        x, edge_index,
        [(np.asarray(W1l), np.asarray(W1r)), (np.asarray(W2l), np.asarray(W2r)),
         (np.asarray(W3l), np.asarray(W3r))],
        [a1, a2, a3])
    b1r = np.broadcast_to(np.asarray(b1, np.float32), (P, SB, F1)).reshape(P, SB * F1).astype(BF16).copy()
    b2r = np.broadcast_to(np.asarray(b2, np.float32), (P, SB, F1)).reshape(P, SB * F1).astype(BF16).copy()
    b3r = np.broadcast_to(np.asarray(b3, np.float32), (P, 2, C3)).reshape(P, 2 * C3).astype(np.float32).copy()
    for c in range(NCORES):
        in_maps[c]["brep1"] = b1r
        in_maps[c]["brep2"] = b2r
        in_maps[c]["brep3"] = b3r

    key = (m.Gtot, m.Ptot, tuple(m.sb_g), tuple(tuple(o) for o in m.sb_cls_off),
           tuple(tuple(pl) for pl in m.sb_pairs), tuple(map(tuple, m.gc_max)))
    if key not in _cache:
        _cache.clear()
        _cache[key] = _build_program(m)
    nc = _cache[key]

    res = run_bass_kernel_spmd(nc, in_maps, core_ids=list(range(NCORES)),
                               trace=_trace, tmpdir=_tmpdir)
    out = np.concatenate([res.results[c]["out_shard"] for c in range(NCORES)], axis=0)
    kernel._last_result = res
    return out


# revision 42
# speedup vs baseline: 1.0261x; 1.0261x over previous
"""3-layer GATv2 (PyG GATv2Conv semantics) on 8 Trainium2 NeuronCores.

Distribution: nodes sharded 12500/core; edges (incl. self-loops) partitioned
by dst core, grouped into 128-dst-node blocks. Per layer:
  phase A: [xl|xr] = h @ [Wl|Wr] for local nodes (PE), rows bf16,
           AllGather -> every core reads all rows from the shared buffer.
  phase B: per superblock of blocks, batched dma_gather of xl[src] only
           (int16 idx; global src rows via src%4 class split). xr[dst] is
           NOT gathered: per slot, the one-hot S [lane, dst] (DVE iota
           compare) is PE-transposed to S_T and a PE matmul S_T^T @ xr_blk
           materializes xr per edge in PSUM. z = xl[src]+xr[dst] (DVE),
           lrelu on Scalar (alpha=0.2), score = reduce(z*a) (DVE),
           w = exp(score) (Scalar), per-block indicator matmul S.T @
           [w*xg | w] accumulates weighted sums + denominators in PSUM.
           Superblock-batched divide/bias/ELU; per-block PE transpose emits
           h_T for the next layer. Layer 3: divide, head-mean, f32 output.
"""
import sys
sys.path.insert(0, "/opt/trn_rl_repo")
import numpy as np
import ml_dtypes

N = 100000
E = 800000
NCORES = 8
SHARD = N // NCORES        # 12500
P = 128
NBLK = (SHARD + P - 1) // P  # 98
SB = 4                      # node blocks per superblock
FIN = 64
H = 4
C1, C3 = 16, 32
F1 = H * C1                # 64
F3 = H * C3                # 128
NEG_SLOPE = 0.2

BF16 = ml_dtypes.bfloat16

_cache = {}


class Meta:
    pass


def _preprocess(edge_index):
    """Sort edges by dst; per (core, block) split by src parity; pad each run
    to x128 (uniform across cores). Group order per superblock:
    class-major, block-minor. Returns per-core idx arrays + graph meta."""
    src = np.concatenate([edge_index[0], np.arange(N, dtype=np.int32)])
    dst = np.concatenate([edge_index[1], np.arange(N, dtype=np.int32)])
    order = np.argsort(dst, kind="stable")
    src_s = src[order].astype(np.int64)
    dst_s = dst[order].astype(np.int64)

    core = dst_s // SHARD
    blk = (dst_s - core * SHARD) // P
    key = core * NBLK + blk
    cnt = np.bincount(key, minlength=NCORES * NBLK).reshape(NCORES, NBLK)
    starts = np.concatenate([[0], np.cumsum(cnt.reshape(-1))])

    # self-edges (src==dst) are handled on-device from local rows; count
    # multiplicity per node and exclude them from the gather classes.
    loop_m = src_s == dst_s
    mult = np.bincount(dst_s[loop_m], minlength=N).astype(np.float32)

    # src classes by direct 32768-row ranges (int16-exact, stride-1 tables);
    # runs kept per (core, sb, class) densely packed (slots may span blocks)
    NCLS = 4
    NSB = (NBLK + SB - 1) // SB
    gc = np.zeros((NCORES, NSB, NCLS), np.int64)
    runs = {}
    for c in range(NCORES):
        for s in range(NSB):
            blo, bhi = s * SB, min(NBLK, (s + 1) * SB)
            i0, i1 = c * NBLK + blo, c * NBLK + bhi
            e0, e1 = starts[i0], starts[i1]
            nl = ~loop_m[e0:e1]
            sr, dr = src_s[e0:e1][nl], dst_s[e0:e1][nl]
            cls = sr >> 15
            for r in range(NCLS):
                sel = cls == r
                runs[(c, s, r)] = (sr[sel], dr[sel])
                gc[c, s, r] = sel.sum()
    Gc = np.maximum(1, -(-gc.max(axis=0) // P))   # [NSB, NCLS] groups per run
    mult_pad = np.zeros((NCORES, NBLK * P), np.float32)
    mult_pad[:, :SHARD] = mult.reshape(NCORES, SHARD)
    multc = mult_pad.reshape(NCORES, NBLK, P).transpose(0, 2, 1).astype(BF16).copy()

    m = Meta()
    m.NCLS = NCLS
    m.NSB = NSB
    m.sb_blocks = [list(range(s * SB, min(NBLK, (s + 1) * SB))) for s in range(NSB)]
    m.sb_cls_off = []  # per sb: [o0..o4] class group (slot) offsets
    m.sb_g = []        # per sb: total slots
    for s in range(NSB):
        offs = [0]
        for r in range(NCLS):
            offs.append(offs[-1] + int(Gc[s, r]))
        m.sb_cls_off.append(offs)
        m.sb_g.append(offs[-1])
    m.SBGmax = max(m.sb_g)
    m.Gtot = sum(m.sb_g)

    # pairs (slot, block) per sb: union over cores of blocks intersecting
    # each slot's dense lane range
    m.sb_pairs = []      # per sb: ordered list of (slot, a)
    m.blk_pairs = []     # per sb: {a: [pair ids]}
    m.slot_pairs = []    # per sb: {slot: [pair ids]}
    for s in range(NSB):
        blo = s * SB
        pairset = set()
        for c in range(NCORES):
            for r in range(NCLS):
                offr = m.sb_cls_off[s][r]
                sr, dr = runs[(c, s, r)]
                blk = (dr - c * SHARD) // P - blo
                for k in range(int(Gc[s, r])):
                    seg = blk[k * P:(k + 1) * P]
                    for a in np.unique(seg):
                        pairset.add((offr + k, int(a)))
        pl = sorted(pairset)
        m.sb_pairs.append(pl)
        bp = {}
        sp = {}
        for pid, (sl, a) in enumerate(pl):
            bp.setdefault(a, []).append(pid)
            sp.setdefault(sl, []).append(pid)
        m.blk_pairs.append(bp)
        m.slot_pairs.append(sp)
    m.sb_p = [len(pl) for pl in m.sb_pairs]
    m.NPmax = max(m.sb_p)
    m.Ptot = sum(m.sb_p)
    sb_poff = np.concatenate([[0], np.cumsum(m.sb_p)])
    m.sb_poff = [int(v) for v in sb_poff]

    # per-core flat src idx arrays in (sb, slot, lane) order; rel per PAIR.
    # pads = -1: the Q7 trims trailing negatives, so padding costs no descs.
    src_idx = np.full((NCORES, m.Gtot * P), -1, np.int16)
    rel = np.full((NCORES, m.Ptot * P), -1.0, np.float32)
    sb_goff = np.concatenate([[0], np.cumsum(m.sb_g)])
    m.sb_goff = [int(v) for v in sb_goff]
    for c in range(NCORES):
        for s in range(NSB):
            blo = s * SB
            base = sb_goff[s] * P
            pbase = m.sb_poff[s] * P
            pl = m.sb_pairs[s]
            pair_id = {pa: i for i, pa in enumerate(pl)}
            for r in range(NCLS):
                offr = m.sb_cls_off[s][r]
                sr, dr = runs[(c, s, r)]
                o = base + offr * P
                src_idx[c, o:o + len(sr)] = sr - (r << 15)
                # pad with idx 0 up to the cross-core max count (kept valid so
                # num_idxs_reg == nonneg-count holds on every core); beyond
                # that stays -1 and the Q7 trims it at zero descriptor cost
                maxcr = ((int(gc[:, s, r].max()) + 127) // 128) * 128
                src_idx[c, o + len(sr):o + maxcr] = 0
                drel = dr - c * SHARD
                blk = drel // P - blo
                for i in range(len(sr)):
                    sl = offr + i // P
                    pid = pair_id[(sl, int(blk[i]))]
                    rel[c, pbase + pid * P + i % P] = drel[i] - (blk[i] + blo) * P
    m.Gc = Gc
    m.gc_max = gc.max(axis=0)   # [NSB, NCLS] per-call valid count (uniform)
    m.multc = multc

    def wrap(a):
        w = a.reshape(NCORES, m.Gtot * P // 16, 16).transpose(0, 2, 1)
        return np.tile(w, (1, 8, 1)).copy()

    m.src_w = wrap(src_idx)
    m.rel_pm = rel.reshape(NCORES, m.Ptot, P).transpose(0, 2, 1).astype(BF16).copy()
    return m


def _build_program(m):
    import concourse.bass as bass
    import concourse.bacc as bacc
    import concourse.tile as tile
    from concourse import mybir, library_config

    bf16, f32, i16 = mybir.dt.bfloat16, mybir.dt.float32, mybir.dt.int16
    AF = mybir.ActivationFunctionType
    OP = mybir.AluOpType
    X = mybir.AxisListType.X

    nc = bacc.Bacc("TRN2", target_bir_lowering=False)

    WI = m.Gtot * P // 16
    xT_d = nc.dram_tensor("xT", [FIN, SHARD], bf16, kind="ExternalInput")
    multb_d = nc.dram_tensor("multb", [P, NBLK], bf16, kind="ExternalInput")
    srcw_d = nc.dram_tensor("srcw", [P, WI], i16, kind="ExternalInput")
    dstrel_d = nc.dram_tensor("dstrel", [P, m.Ptot], bf16, kind="ExternalInput")
    iota_d = nc.dram_tensor("iota", [P, m.NPmax * P], bf16, kind="ExternalInput")
    W_d = [nc.dram_tensor(f"W{l}", [FIN, 2 * (F1 if l < 3 else F3)], bf16, kind="ExternalInput") for l in (1, 2, 3)]
    arep_d = [nc.dram_tensor(f"arep{l}", [P, m.SBGmax * (F1 if l < 3 else F3)], bf16, kind="ExternalInput") for l in (1, 2, 3)]
    brep_d = [nc.dram_tensor("brep1", [P, SB * F1], bf16, kind="ExternalInput"),
              nc.dram_tensor("brep2", [P, SB * F1], bf16, kind="ExternalInput"),
              nc.dram_tensor("brep3", [P, 2 * C3], f32, kind="ExternalInput")]
    ident_d = nc.dram_tensor("ident", [P, P], bf16, kind="ExternalInput")
    out_d = nc.dram_tensor("out_shard", [SHARD, C3], f32, kind="ExternalOutput")

    def internal(name, shape, dt, shared=False):
        return nc.dram_tensor(name, shape, dt, kind="Internal",
                              addr_space="Shared" if shared else "Local")

    hT_next = [internal(f"hT{l}", [F1, SHARD], bf16) for l in (1, 2)]
    xlr_sh = [internal(f"xlrsh{l}", [SHARD, 2 * (F1 if l < 3 else F3)], bf16) for l in (1, 2, 3)]
    xlr_rows_cc = [internal(f"xlrrowscc{l}", [N, 2 * (F1 if l < 3 else F3)], bf16, shared=True) for l in (1, 2, 3)]

    RG = [list(range(NCORES))]

    with tile.TileContext(nc) as tc:
        nc.gpsimd.load_library(library_config.mlp)
        with tc.tile_pool(name="const", bufs=1) as cpool, \
             tc.tile_pool(name="work", bufs=2) as wpool, \
             tc.tile_pool(name="mmA", bufs=3) as apool, \
             tc.tile_pool(name="tail", bufs=2) as tpool, \
             tc.tile_pool(name="psAT", bufs=2, space="PSUM") as ppAT, \
             tc.tile_pool(name="psB", bufs=2, space="PSUM") as ppB, \
             tc.tile_pool(name="psST", bufs=2, space="PSUM") as ppST, \
             tc.tile_pool(name="psXR", bufs=2, space="PSUM") as ppXR:

            srcw = cpool.tile([P, WI], i16)
            dstrel = cpool.tile([P, m.Ptot], bf16)
            iota = cpool.tile([P, m.NPmax * P], bf16)
            ident = cpool.tile([P, P], bf16)
            multb = cpool.tile([P, NBLK], bf16)
            for t, d in [(srcw, srcw_d), (dstrel, dstrel_d),
                         (iota, iota_d), (ident, ident_d), (multb, multb_d)]:
                nc.sync.dma_start(t[:], d[:])
            W_sb, arep_sb, brep_sb = [], [], []
            for li in range(3):
                Fl = F1 if li < 2 else F3
                w = cpool.tile([FIN, 2 * Fl], bf16, tag=f"W{li}")
                nc.sync.dma_start(w[:], W_d[li][:])
                W_sb.append(w)
                a = cpool.tile([P, m.SBGmax * Fl], bf16, tag=f"arep{li}")
                nc.sync.dma_start(a[:], arep_d[li][:])
                arep_sb.append(a)
                b = cpool.tile([P, SB * F1 if li < 2 else 2 * C3],
                               bf16 if li < 2 else f32, tag=f"brep{li}")
                nc.sync.dma_start(b[:], brep_d[li][:])
                brep_sb.append(b)
            # persistent ping-pong gather buffers, memset once so pad lanes
            # (skipped by the Q7's trailing-negative trim) stay finite
            xg0 = cpool.tile([P, m.SBGmax, F3], bf16, tag="xg0")
            xg1 = cpool.tile([P, m.SBGmax, F3], bf16, tag="xg1")
            nc.vector.memset(xg0[:], 0.0)
            nc.vector.memset(xg1[:], 0.0)
            xg_pp = [xg0, xg1]
            # resident xT for phase A
            xTr = cpool.tile([FIN, SHARD], bf16, tag="xTr")
            nc.sync.dma_start(xTr[:], xT_d[:])

            for li in range(3):
                l3 = (li == 2)
                Fl = F3 if l3 else F1
                Cl = C3 if l3 else C1
                FE = 2 * Fl            # row width of xlr tensors

                # ---- phase A (layer 1 only; later layers fused in B tails) ----
                if li == 0:
                    for t in range(NBLK):
                        n0 = t * P
                        mm = min(P, SHARD - n0)
                        psA = ppAT.tile([P, 2 * F3], f32, tag="psAT", space="PSUM")
                        nc.tensor.matmul(psA[:mm, :FE], lhsT=xTr[:, n0:n0 + mm],
                                         rhs=W_sb[li][:], start=True, stop=True)
                        xlr = apool.tile([P, 2 * F3], bf16, tag="xlr")
                        nc.scalar.copy(xlr[:mm, :FE], psA[:mm, :FE])
                        nc.sync.dma_start(xlr_sh[li][n0:n0 + mm, :], xlr[:mm, :FE])

                nc.gpsimd.collective_compute(
                    "AllGather", mybir.AluOpType.bypass, replica_groups=RG,
                    ins=[xlr_sh[li][:]], outs=[xlr_rows_cc[li][:]])

                # gather source views: 4 direct-range classes (idx = src - r*32768)
                if not l3:
                    src_tabs = [xlr_rows_cc[li][r * 32768:, :] for r in range(m.NCLS)]
                    GELEM, GSTEP = FE, FE
                else:
                    src_tabs = [xlr_rows_cc[li][r * 32768:, :F3] for r in range(m.NCLS)]
                    GELEM, GSTEP = F3, FE

                # ---- phase B ----
                for s in range(m.NSB):
                    SG = m.sb_g[s]
                    go = m.sb_goff[s]
                    po = m.sb_poff[s]
                    NP = m.sb_p[s]
                    pairs = m.sb_pairs[s]
                    spairs = m.slot_pairs[s]
                    wo = go * P // 16
                    blocks = m.sb_blocks[s]
                    nblk = len(blocks)
                    xg = xg_pp[s % 2]
                    offs = m.sb_cls_off[s]
                    for r in range(m.NCLS):
                        nr = (offs[r + 1] - offs[r]) * P
                        nvalid = min(nr, ((int(m.gc_max[s, r]) + 127) // 128) * 128)
                        if nr == 0 or nvalid == 0:
                            continue
                        nc.gpsimd.dma_gather(
                            out_ap=xg[:, offs[r]:offs[r + 1], :GELEM], in_ap=src_tabs[r],
                            idxs_ap=srcw[:, wo + offs[r] * 8:wo + offs[r + 1] * 8],
                            num_idxs=nr, num_idxs_reg=nvalid, elem_size=GELEM,
                            elem_step=GSTEP, single_packet=False)

                    # local [xl|xr] rows per block: [P, SB, FE]
                    xrb = wpool.tile([P, SB, 2 * F3], bf16, tag="xrb")
                    if blocks[-1] * P + P > SHARD:
                        nc.vector.memset(xrb[:], 0.0)
                    for a, b in enumerate(blocks):
                        n0 = b * P
                        mm = min(P, SHARD - n0)
                        nc.sync.dma_start(xrb[:mm, a, :FE],
                                          xlr_sh[li][n0:n0 + mm, :])

                    # one-hot S [lane, dst-in-block] per PAIR (slot, block)
                    S = wpool.tile([P, m.NPmax, P], bf16, tag="S")
                    nc.vector.tensor_tensor(
                        out=S[:, :NP, :],
                        in0=iota[:, :NP * P].rearrange("p (g n) -> p g n", g=NP),
                        in1=dstrel[:, po:po + NP].to_broadcast([P, NP, P]),
                        op=OP.is_equal)

                    # z = xl[src] + xr[dst]; xr via PE: S_T = transpose(S),
                    # then psum_xr[slot] += S_T^T @ xr_blk over the slot's pairs
                    zl = wpool.tile([P, m.SBGmax, F3 if l3 else F1], bf16, tag="zl")
                    psxr_t = {}
                    nqp = (NP + 3) // 4
                    for q in range(nqp):
                        w4 = min(4, NP - 4 * q)
                        psST = ppST.tile([P, 4, P], bf16, tag="psST", space="PSUM")
                        for j in range(w4):
                            nc.tensor.transpose(psST[:, j, :], S[:, 4 * q + j, :], ident[:])
                        STt = wpool.tile([P, 4, P], bf16, tag="STt")
                        if q % 2 == 0:
                            nc.scalar.copy(STt[:, :w4, :], psST[:, :w4, :])
                        else:
                            nc.vector.tensor_copy(STt[:, :w4, :], psST[:, :w4, :])
                        for j in range(w4):
                            pid = 4 * q + j
                            sl, a = pairs[pid]
                            sq = sl // 4
                            if sq not in psxr_t:
                                t_xr = ppXR.tile([P, 4, F3], f32, tag="psXR", space="PSUM")
                                psxr_t[sq] = t_xr
                            plist = spairs[sl]
                            nc.tensor.matmul(psxr_t[sq][:, sl % 4, :Fl],
                                             lhsT=STt[:, j, :],
                                             rhs=xrb[:, a, Fl:FE],
                                             start=(pid == plist[0]),
                                             stop=(pid == plist[-1]))
                            if pid == NP - 1 or pairs[pid + 1][0] // 4 > sq:
                                w4s = min(4, SG - sq * 4)
                                nc.vector.tensor_tensor(
                                    out=zl[:, sq * 4:sq * 4 + w4s, :Fl],
                                    in0=xg[:, sq * 4:sq * 4 + w4s, :Fl],
                                    in1=psxr_t[sq][:, :w4s, :Fl], op=OP.add)
                                del psxr_t[sq]

                    # lrelu = max(0.2 z, z) on DVE, then *att, reduce -> score
                    nc.vector.scalar_tensor_tensor(
                        out=zl[:, :SG, :Fl], in0=zl[:, :SG, :Fl], scalar=NEG_SLOPE,
                        in1=zl[:, :SG, :Fl], op0=OP.mult, op1=OP.max)
                    nc.vector.tensor_tensor(
                        out=zl[:, :SG, :Fl], in0=zl[:, :SG, :Fl],
                        in1=arep_sb[li][:, :SG * Fl].rearrange("p (g f) -> p g f", g=SG),
                        op=OP.mult)
                    score = wpool.tile([P, m.SBGmax * H], f32, tag="score")
                    nc.vector.tensor_reduce(
                        out=score[:, :SG * H],
                        in_=zl[:, :SG, :Fl].rearrange("p g (h c) -> p g h c", h=H),
                        axis=X, op=OP.add)
                    wlhs = wpool.tile([P, m.SBGmax, F3 + H], bf16, tag="wlhs")
                    nc.scalar.activation(
                        wlhs[:, :SG, :Fl].rearrange("p g (h c) -> p g h c", h=H),
                        score[:, :SG * H].rearrange("p (g h) -> p g h", g=SG).to_broadcast([P, SG, H, Cl]),
                        AF.Exp)
                    nc.scalar.activation(
                        wlhs[:, :SG, Fl:Fl + H],
                        score[:, :SG * H].rearrange("p (g h) -> p g h", g=SG),
                        AF.Exp)
                    nc.vector.tensor_tensor(out=wlhs[:, :SG, :Fl],
                                            in0=xg[:, :SG, :Fl],
                                            in1=wlhs[:, :SG, :Fl], op=OP.mult)

                    # self-edge contribution from local rows (no gather):
                    # zs = xl[d]+xr[d]; wx = mult * [exp(a.lrelu(zs)) * xl | exp]
                    zs = tpool.tile([P, SB, F3], bf16, tag="zs")
                    nc.vector.tensor_tensor(out=zs[:, :nblk, :Fl],
                                            in0=xrb[:, :nblk, :Fl],
                                            in1=xrb[:, :nblk, Fl:FE], op=OP.add)
                    nc.vector.scalar_tensor_tensor(
                        out=zs[:, :nblk, :Fl], in0=zs[:, :nblk, :Fl],
                        scalar=NEG_SLOPE, in1=zs[:, :nblk, :Fl],
                        op0=OP.mult, op1=OP.max)
                    nc.vector.tensor_tensor(
                        out=zs[:, :nblk, :Fl], in0=zs[:, :nblk, :Fl],
                        in1=arep_sb[li][:, :nblk * Fl].rearrange("p (a f) -> p a f", a=nblk),
                        op=OP.mult)
                    ssc = tpool.tile([P, SB * H], f32, tag="ssc")
                    nc.vector.tensor_reduce(
                        out=ssc[:, :nblk * H],
                        in_=zs[:, :nblk, :Fl].rearrange("p a (h c) -> p a h c", h=H),
                        axis=X, op=OP.add)
                    wx = tpool.tile([P, SB, F3 + H], bf16, tag="wx")
                    nc.scalar.activation(
                        wx[:, :nblk, :Fl].rearrange("p a (h c) -> p a h c", h=H),
                        ssc[:, :nblk * H].rearrange("p (a h) -> p a h", a=nblk).to_broadcast([P, nblk, H, Cl]),
                        AF.Exp)
                    nc.scalar.activation(
                        wx[:, :nblk, Fl:Fl + H],
                        ssc[:, :nblk * H].rearrange("p (a h) -> p a h", a=nblk),
                        AF.Exp)
                    nc.vector.tensor_tensor(out=wx[:, :nblk, :Fl],
                                            in0=xrb[:, :nblk, :Fl],
                                            in1=wx[:, :nblk, :Fl], op=OP.mult)
                    b0 = blocks[0]
                    nc.vector.tensor_tensor(
                        out=wx[:, :nblk, :Fl + H], in0=wx[:, :nblk, :Fl + H],
                        in1=multb[:, b0:b0 + nblk, None].to_broadcast([P, nblk, Fl + H]),
                        op=OP.mult)

                    # aggregation: per block, accumulate S.T @ [w*xl | w]
                    bpairs = m.blk_pairs[s]
                    if not l3:
                        psB = ppB.tile([P, SB, F1 + H], f32, tag="psB", space="PSUM")
                        for a, b in enumerate(blocks):
                            plist = bpairs[a]
                            for i, pid in enumerate(plist):
                                nc.tensor.matmul(psB[:, a, :Fl + H], lhsT=S[:, pid, :],
                                                 rhs=wlhs[:, pairs[pid][0], :Fl + H],
                                                 start=(i == 0), stop=(i == len(plist) - 1))
                        nc.vector.tensor_tensor(
                            out=psB[:, :nblk, :Fl + H], in0=psB[:, :nblk, :Fl + H],
                            in1=wx[:, :nblk, :Fl + H], op=OP.add)
                        # superblock-batched tail
                        rec = tpool.tile([P, SB, H], f32, tag="rec")
                        nc.vector.reciprocal(rec[:, :nblk, :], psB[:, :nblk, Fl:Fl + H])
                        hb = tpool.tile([P, SB, F1], bf16, tag="hb")
                        nc.vector.tensor_tensor(
                            out=hb[:, :nblk, :].rearrange("p a (h c) -> p a h c", h=H),
                            in0=psB[:, :nblk, :Fl].rearrange("p a (h c) -> p a h c", h=H),
                            in1=rec[:, :nblk, :, None].to_broadcast([P, nblk, H, Cl]),
                            op=OP.mult)
                        nc.vector.tensor_tensor(
                            out=hb[:, :nblk, :],
                            in0=hb[:, :nblk, :],
                            in1=brep_sb[li][:, :nblk * F1].rearrange("p (a f) -> p a f", a=nblk),
                            op=OP.add)
                        rp = tpool.tile([P, SB, F1], bf16, tag="rp")
                        nc.scalar.activation(rp[:, :nblk, :], hb[:, :nblk, :], AF.Relu)
                        xm = tpool.tile([P, SB, F1], bf16, tag="xm")
                        nc.vector.tensor_scalar_min(out=xm[:, :nblk, :],
                                                    in0=hb[:, :nblk, :], scalar1=0.0)
                        ex = tpool.tile([P, SB, F1], f32, tag="ex")
                        nc.scalar.activation(ex[:, :nblk, :], xm[:, :nblk, :], AF.Exp)
                        ho = tpool.tile([P, SB, F1], bf16, tag="ho")
                        nc.vector.scalar_tensor_tensor(
                            out=ho[:, :nblk, :], in0=ex[:, :nblk, :], scalar=-1.0,
                            in1=rp[:, :nblk, :], op0=OP.add, op1=OP.add)
                        for a, b in enumerate(blocks):
                            n0 = b * P
                            mm = min(P, SHARD - n0)
                            psT = ppAT.tile([F1, P], bf16, tag="psAT", space="PSUM")
                            nc.tensor.transpose(psT[:, :mm], ho[:mm, a, :], ident[:mm, :mm])
                            hTn = tpool.tile([F1, P], bf16, tag="hTn")
                            nc.scalar.copy(hTn[:, :mm], psT[:, :mm])
                            # fused phase A of the next layer
                            FEn = 2 * (F1 if li == 0 else F3)
                            psA = ppAT.tile([P, 2 * F3], f32, tag="psAT", space="PSUM")
                            nc.tensor.matmul(psA[:mm, :FEn], lhsT=hTn[:, :mm],
                                             rhs=W_sb[li + 1][:], start=True, stop=True)
                            xlr = apool.tile([P, 2 * F3], bf16, tag="xlr")
                            nc.scalar.copy(xlr[:mm, :FEn], psA[:mm, :FEn])
                            nc.sync.dma_start(xlr_sh[li + 1][n0:n0 + mm, :], xlr[:mm, :FEn])
                    else:
                        for half in range((nblk + 1) // 2):
                            hb0 = 2 * half
                            hblocks = blocks[hb0:hb0 + 2]
                            nh = len(hblocks)
                            psB3 = ppB.tile([P, 2, F3 + H], f32, tag="psB", space="PSUM")
                            for a, b in enumerate(hblocks):
                                plist = bpairs[hb0 + a]
                                for i, pid in enumerate(plist):
                                    nc.tensor.matmul(psB3[:, a, :F3 + H], lhsT=S[:, pid, :],
                                                     rhs=wlhs[:, pairs[pid][0], :F3 + H],
                                                     start=(i == 0), stop=(i == len(plist) - 1))
                            nc.vector.tensor_tensor(
                                out=psB3[:, :nh, :F3 + H], in0=psB3[:, :nh, :F3 + H],
                                in1=wx[:, hb0:hb0 + nh, :F3 + H], op=OP.add)
                            rec = tpool.tile([P, 2, H], f32, tag="rec3")
                            nc.vector.reciprocal(rec[:, :nh, :], psB3[:, :nh, F3:F3 + H])
                            o3 = tpool.tile([P, 2, F3], f32, tag="o3")
                            nc.vector.tensor_tensor(
                                out=o3[:, :nh, :].rearrange("p a (h c) -> p a h c", h=H),
                                in0=psB3[:, :nh, :F3].rearrange("p a (h c) -> p a h c", h=H),
                                in1=rec[:, :nh, :, None].to_broadcast([P, nh, H, Cl]),
                                op=OP.mult)
                            m01 = tpool.tile([P, 2, C3], f32, tag="m01")
                            nc.vector.tensor_tensor(out=m01[:, :nh, :], in0=o3[:, :nh, 0:C3],
                                                    in1=o3[:, :nh, C3:2 * C3], op=OP.add)
                            m23 = tpool.tile([P, 2, C3], f32, tag="m23")
                            nc.vector.tensor_tensor(out=m23[:, :nh, :], in0=o3[:, :nh, 2 * C3:3 * C3],
                                                    in1=o3[:, :nh, 3 * C3:4 * C3], op=OP.add)
                            ms = tpool.tile([P, 2, C3], f32, tag="ms")
                            nc.vector.tensor_tensor(out=ms[:, :nh, :], in0=m01[:, :nh, :],
                                                    in1=m23[:, :nh, :], op=OP.add)
                            of = tpool.tile([P, 2, C3], f32, tag="of")
                            nc.vector.scalar_tensor_tensor(
                                out=of[:, :nh, :], in0=ms[:, :nh, :], scalar=0.25,
                                in1=brep_sb[2][:, :nh * C3].rearrange("p (a c) -> p a c", a=nh),
                                op0=OP.mult, op1=OP.add)
                            for a, b in enumerate(hblocks):
                                n0 = b * P
                                mm = min(P, SHARD - n0)
                                nc.sync.dma_start(out_d[n0:n0 + mm, :], of[:mm, a, :])

    nc.compile()
    return nc


def _prep_inputs(x, edge_index, Ws, atts):
    m = _preprocess(edge_index)
    ident = np.eye(P, dtype=np.float32).astype(BF16)
    iota = np.broadcast_to(np.arange(P, dtype=np.float32), (P, m.NPmax, P)) \
        .reshape(P, m.NPmax * P).astype(BF16).copy()
    common = {"ident": ident, "iota": iota}
    for li, ((Wl, Wr), a) in enumerate(zip(Ws, atts)):
        Fl = Wl.shape[1]
        common[f"W{li + 1}"] = np.concatenate([Wl, Wr], axis=1).astype(BF16)
        a_flat = np.asarray(a).reshape(Fl).astype(np.float32)
        common[f"arep{li + 1}"] = np.broadcast_to(a_flat, (P, m.SBGmax, Fl)) \
            .reshape(P, m.SBGmax * Fl).astype(BF16).copy()
    in_maps = []
    for c in range(NCORES):
        d = dict(common)
        d["xT"] = x[c * SHARD:(c + 1) * SHARD].T.astype(BF16).copy()
        d["srcw"] = m.src_w[c]
        d["dstrel"] = m.rel_pm[c]
        d["multb"] = m.multc[c]
        in_maps.append(d)
    return in_maps, m


def kernel(x, edge_index, W1l, W1r, a1, b1, W2l, W2r, a2, b2, W3l, W3r, a3, b3,
           _trace=False, _tmpdir=None):
    from concourse.bass_utils import run_bass_kernel_spmd

    x = np.asarray(x, dtype=np.float32)
    edge_index = np.asarray(edge_index, dtype=np.int32)
    in_maps, m = _prep_inputs(
        x, edge_index,
        [(np.asarray(W1l), np.asarray(W1r)), (np.asarray(W2l), np.asarray(W2r)),
         (np.asarray(W3l), np.asarray(W3r))],
        [a1, a2, a3])
    b1r = np.broadcast_to(np.asarray(b1, np.float32), (P, SB, F1)).reshape(P, SB * F1).astype(BF16).copy()
    b2r = np.broadcast_to(np.asarray(b2, np.float32), (P, SB, F1)).reshape(P, SB * F1).astype(BF16).copy()
    b3r = np.broadcast_to(np.asarray(b3, np.float32), (P, 2, C3)).reshape(P, 2 * C3).astype(np.float32).copy()
    for c in range(NCORES):
        in_maps[c]["brep1"] = b1r
        in_maps[c]["brep2"] = b2r
        in_maps[c]["brep3"] = b3r

    key = (m.Gtot, m.Ptot, tuple(m.sb_g), tuple(tuple(o) for o in m.sb_cls_off),
           tuple(tuple(pl) for pl in m.sb_pairs), tuple(map(tuple, m.gc_max)))
    if key not in _cache:
        _cache.clear()
        _cache[key] = _build_program(m)
    nc = _cache[key]

    res = run_bass_kernel_spmd(nc, in_maps, core_ids=list(range(NCORES)),
                               trace=_trace, tmpdir=_tmpdir)
    out = np.concatenate([res.results[c]["out_shard"] for c in range(NCORES)], axis=0)
    kernel._last_result = res
    return out


# revision 44
# speedup vs baseline: 1.0533x; 1.0265x over previous
"""3-layer GATv2 (PyG GATv2Conv semantics) on 8 Trainium2 NeuronCores.

Distribution: nodes sharded 12500/core; edges (incl. self-loops) partitioned
by dst core, grouped into 128-dst-node blocks. Per layer:
  phase A: [xl|xr] = h @ [Wl|Wr] for local nodes (PE), rows bf16,
           AllGather -> every core reads all rows from the shared buffer.
  phase B: per superblock of blocks, batched dma_gather of xl[src] only
           (int16 idx; global src rows via src%4 class split). xr[dst] is
           NOT gathered: per slot, the one-hot S [lane, dst] (DVE iota
           compare) is PE-transposed to S_T and a PE matmul S_T^T @ xr_blk
           materializes xr per edge in PSUM. z = xl[src]+xr[dst] (DVE),
           lrelu on Scalar (alpha=0.2), score = reduce(z*a) (DVE),
           w = exp(score) (Scalar), per-block indicator matmul S.T @
           [w*xg | w] accumulates weighted sums + denominators in PSUM.
           Superblock-batched divide/bias/ELU; per-block PE transpose emits
           h_T for the next layer. Layer 3: divide, head-mean, f32 output.
"""
import sys
sys.path.insert(0, "/opt/trn_rl_repo")
import numpy as np
import ml_dtypes

N = 100000
E = 800000
NCORES = 8
SHARD = N // NCORES        # 12500
P = 128
NBLK = (SHARD + P - 1) // P  # 98
SB = 4                      # node blocks per superblock
FIN = 64
H = 4
C1, C3 = 16, 32
F1 = H * C1                # 64
F3 = H * C3                # 128
NEG_SLOPE = 0.2

BF16 = ml_dtypes.bfloat16

_cache = {}


class Meta:
    pass


def _preprocess(edge_index):
    """Sort edges by dst; per (core, block) split by src parity; pad each run
    to x128 (uniform across cores). Group order per superblock:
    class-major, block-minor. Returns per-core idx arrays + graph meta."""
    src = np.concatenate([edge_index[0], np.arange(N, dtype=np.int32)])
    dst = np.concatenate([edge_index[1], np.arange(N, dtype=np.int32)])
    order = np.argsort(dst, kind="stable")
    src_s = src[order].astype(np.int64)
    dst_s = dst[order].astype(np.int64)

    core = dst_s // SHARD
    blk = (dst_s - core * SHARD) // P
    key = core * NBLK + blk
    cnt = np.bincount(key, minlength=NCORES * NBLK).reshape(NCORES, NBLK)
    starts = np.concatenate([[0], np.cumsum(cnt.reshape(-1))])

    # self-edges (src==dst) are handled on-device from local rows; count
    # multiplicity per node and exclude them from the gather classes.
    loop_m = src_s == dst_s
    mult = np.bincount(dst_s[loop_m], minlength=N).astype(np.float32)

    # src classes by direct 32768-row ranges (int16-exact, stride-1 tables);
    # runs kept per (core, sb, class) densely packed (slots may span blocks)
    NCLS = 4
    NSB = (NBLK + SB - 1) // SB
    gc = np.zeros((NCORES, NSB, NCLS), np.int64)
    runs = {}
    for c in range(NCORES):
        for s in range(NSB):
            blo, bhi = s * SB, min(NBLK, (s + 1) * SB)
            i0, i1 = c * NBLK + blo, c * NBLK + bhi
            e0, e1 = starts[i0], starts[i1]
            nl = ~loop_m[e0:e1]
            sr, dr = src_s[e0:e1][nl], dst_s[e0:e1][nl]
            # chunked-AG row layout: [c0 first-half | c1 fh | ... | c0 second-half | ...]
            HS = SHARD // 2
            lr = sr % SHARD
            sr = np.where(lr < HS, (sr // SHARD) * HS + lr,
                          HS * NCORES + (sr // SHARD) * HS + (lr - HS))
            cls = sr >> 15
            for r in range(NCLS):
                sel = cls == r
                runs[(c, s, r)] = (sr[sel], dr[sel])
                gc[c, s, r] = sel.sum()
    Gc = np.maximum(1, -(-gc.max(axis=0) // P))   # [NSB, NCLS] groups per run
    mult_pad = np.zeros((NCORES, NBLK * P), np.float32)
    mult_pad[:, :SHARD] = mult.reshape(NCORES, SHARD)
    multc = mult_pad.reshape(NCORES, NBLK, P).transpose(0, 2, 1).astype(BF16).copy()

    m = Meta()
    m.NCLS = NCLS
    m.NSB = NSB
    m.sb_blocks = [list(range(s * SB, min(NBLK, (s + 1) * SB))) for s in range(NSB)]
    m.sb_cls_off = []  # per sb: [o0..o4] class group (slot) offsets
    m.sb_g = []        # per sb: total slots
    for s in range(NSB):
        offs = [0]
        for r in range(NCLS):
            offs.append(offs[-1] + int(Gc[s, r]))
        m.sb_cls_off.append(offs)
        m.sb_g.append(offs[-1])
    m.SBGmax = max(m.sb_g)
    m.Gtot = sum(m.sb_g)

    # pairs (slot, block) per sb: union over cores of blocks intersecting
    # each slot's dense lane range
    m.sb_pairs = []      # per sb: ordered list of (slot, a)
    m.blk_pairs = []     # per sb: {a: [pair ids]}
    m.slot_pairs = []    # per sb: {slot: [pair ids]}
    for s in range(NSB):
        blo = s * SB
        pairset = set()
        for c in range(NCORES):
            for r in range(NCLS):
                offr = m.sb_cls_off[s][r]
                sr, dr = runs[(c, s, r)]
                blk = (dr - c * SHARD) // P - blo
                for k in range(int(Gc[s, r])):
                    seg = blk[k * P:(k + 1) * P]
                    for a in np.unique(seg):
                        pairset.add((offr + k, int(a)))
        pl = sorted(pairset)
        m.sb_pairs.append(pl)
        bp = {}
        sp = {}
        for pid, (sl, a) in enumerate(pl):
            bp.setdefault(a, []).append(pid)
            sp.setdefault(sl, []).append(pid)
        m.blk_pairs.append(bp)
        m.slot_pairs.append(sp)
    m.sb_p = [len(pl) for pl in m.sb_pairs]
    m.NPmax = max(m.sb_p)
    m.Ptot = sum(m.sb_p)
    sb_poff = np.concatenate([[0], np.cumsum(m.sb_p)])
    m.sb_poff = [int(v) for v in sb_poff]

    # per-core flat src idx arrays in (sb, slot, lane) order; rel per PAIR.
    # pads = -1: the Q7 trims trailing negatives, so padding costs no descs.
    src_idx = np.full((NCORES, m.Gtot * P), -1, np.int16)
    rel = np.full((NCORES, m.Ptot * P), -1.0, np.float32)
    sb_goff = np.concatenate([[0], np.cumsum(m.sb_g)])
    m.sb_goff = [int(v) for v in sb_goff]
    for c in range(NCORES):
        for s in range(NSB):
            blo = s * SB
            base = sb_goff[s] * P
            pbase = m.sb_poff[s] * P
            pl = m.sb_pairs[s]
            pair_id = {pa: i for i, pa in enumerate(pl)}
            for r in range(NCLS):
                offr = m.sb_cls_off[s][r]
                sr, dr = runs[(c, s, r)]
                o = base + offr * P
                src_idx[c, o:o + len(sr)] = sr - (r << 15)
                # pad with idx 0 up to the cross-core max count (kept valid so
                # num_idxs_reg == nonneg-count holds on every core); beyond
                # that stays -1 and the Q7 trims it at zero descriptor cost
                maxcr = ((int(gc[:, s, r].max()) + 127) // 128) * 128
                src_idx[c, o + len(sr):o + maxcr] = 0
                drel = dr - c * SHARD
                blk = drel // P - blo
                for i in range(len(sr)):
                    sl = offr + i // P
                    pid = pair_id[(sl, int(blk[i]))]
                    rel[c, pbase + pid * P + i % P] = drel[i] - (blk[i] + blo) * P
    m.Gc = Gc
    m.gc_max = gc.max(axis=0)   # [NSB, NCLS] per-call valid count (uniform)
    m.multc = multc

    def wrap(a):
        w = a.reshape(NCORES, m.Gtot * P // 16, 16).transpose(0, 2, 1)
        return np.tile(w, (1, 8, 1)).copy()

    m.src_w = wrap(src_idx)
    m.rel_pm = rel.reshape(NCORES, m.Ptot, P).transpose(0, 2, 1).astype(BF16).copy()
    return m


def _build_program(m):
    import concourse.bass as bass
    import concourse.bacc as bacc
    import concourse.tile as tile
    from concourse import mybir, library_config

    bf16, f32, i16 = mybir.dt.bfloat16, mybir.dt.float32, mybir.dt.int16
    AF = mybir.ActivationFunctionType
    OP = mybir.AluOpType
    X = mybir.AxisListType.X

    nc = bacc.Bacc("TRN2", target_bir_lowering=False)

    WI = m.Gtot * P // 16
    xT_d = nc.dram_tensor("xT", [FIN, SHARD], bf16, kind="ExternalInput")
    multb_d = nc.dram_tensor("multb", [P, NBLK], bf16, kind="ExternalInput")
    srcw_d = nc.dram_tensor("srcw", [P, WI], i16, kind="ExternalInput")
    dstrel_d = nc.dram_tensor("dstrel", [P, m.Ptot], bf16, kind="ExternalInput")
    iota_d = nc.dram_tensor("iota", [P, m.NPmax * P], bf16, kind="ExternalInput")
    W_d = [nc.dram_tensor(f"W{l}", [FIN, 2 * (F1 if l < 3 else F3)], bf16, kind="ExternalInput") for l in (1, 2, 3)]
    arep_d = [nc.dram_tensor(f"arep{l}", [P, m.SBGmax * (F1 if l < 3 else F3)], bf16, kind="ExternalInput") for l in (1, 2, 3)]
    brep_d = [nc.dram_tensor("brep1", [P, SB * F1], bf16, kind="ExternalInput"),
              nc.dram_tensor("brep2", [P, SB * F1], bf16, kind="ExternalInput"),
              nc.dram_tensor("brep3", [P, 2 * C3], f32, kind="ExternalInput")]
    ident_d = nc.dram_tensor("ident", [P, P], bf16, kind="ExternalInput")
    out_d = nc.dram_tensor("out_shard", [SHARD, C3], f32, kind="ExternalOutput")

    def internal(name, shape, dt, shared=False):
        return nc.dram_tensor(name, shape, dt, kind="Internal",
                              addr_space="Shared" if shared else "Local")

    hT_next = [internal(f"hT{l}", [F1, SHARD], bf16) for l in (1, 2)]
    xlr_sh = [internal(f"xlrsh{l}", [SHARD, 2 * (F1 if l < 3 else F3)], bf16) for l in (1, 2, 3)]
    xlr_rows_cc = [internal(f"xlrrowscc{l}", [N, 2 * (F1 if l < 3 else F3)], bf16, shared=True) for l in (1, 2, 3)]

    RG = [list(range(NCORES))]

    with tile.TileContext(nc) as tc:
        nc.gpsimd.load_library(library_config.mlp)
        with tc.tile_pool(name="const", bufs=1) as cpool, \
             tc.tile_pool(name="work", bufs=2) as wpool, \
             tc.tile_pool(name="mmA", bufs=3) as apool, \
             tc.tile_pool(name="tail", bufs=2) as tpool, \
             tc.tile_pool(name="psAT", bufs=2, space="PSUM") as ppAT, \
             tc.tile_pool(name="psB", bufs=2, space="PSUM") as ppB, \
             tc.tile_pool(name="psST", bufs=2, space="PSUM") as ppST, \
             tc.tile_pool(name="psXR", bufs=2, space="PSUM") as ppXR:

            srcw = cpool.tile([P, WI], i16)
            dstrel = cpool.tile([P, m.Ptot], bf16)
            iota = cpool.tile([P, m.NPmax * P], bf16)
            ident = cpool.tile([P, P], bf16)
            multb = cpool.tile([P, NBLK], bf16)
            for t, d in [(srcw, srcw_d), (dstrel, dstrel_d),
                         (iota, iota_d), (ident, ident_d), (multb, multb_d)]:
                nc.sync.dma_start(t[:], d[:])
            W_sb, arep_sb, brep_sb = [], [], []
            for li in range(3):
                Fl = F1 if li < 2 else F3
                w = cpool.tile([FIN, 2 * Fl], bf16, tag=f"W{li}")
                nc.sync.dma_start(w[:], W_d[li][:])
                W_sb.append(w)
                a = cpool.tile([P, m.SBGmax * Fl], bf16, tag=f"arep{li}")
                nc.sync.dma_start(a[:], arep_d[li][:])
                arep_sb.append(a)
                b = cpool.tile([P, SB * F1 if li < 2 else 2 * C3],
                               bf16 if li < 2 else f32, tag=f"brep{li}")
                nc.sync.dma_start(b[:], brep_d[li][:])
                brep_sb.append(b)
            # persistent ping-pong gather buffers, memset once so pad lanes
            # (skipped by the Q7's trailing-negative trim) stay finite
            xg0 = cpool.tile([P, m.SBGmax, F3], bf16, tag="xg0")
            xg1 = cpool.tile([P, m.SBGmax, F3], bf16, tag="xg1")
            nc.vector.memset(xg0[:], 0.0)
            nc.vector.memset(xg1[:], 0.0)
            xg_pp = [xg0, xg1]
            # resident xT for phase A
            xTr = cpool.tile([FIN, SHARD], bf16, tag="xTr")
            nc.sync.dma_start(xTr[:], xT_d[:])

            for li in range(3):
                l3 = (li == 2)
                Fl = F3 if l3 else F1
                Cl = C3 if l3 else C1
                FE = 2 * Fl            # row width of xlr tensors

                # ---- phase A (layer 1 only; later layers fused in B tails) ----
                if li == 0:
                    for t in range(NBLK):
                        n0 = t * P
                        mm = min(P, SHARD - n0)
                        psA = ppAT.tile([P, 2 * F3], f32, tag="psAT", space="PSUM")
                        nc.tensor.matmul(psA[:mm, :FE], lhsT=xTr[:, n0:n0 + mm],
                                         rhs=W_sb[li][:], start=True, stop=True)
                        xlr = apool.tile([P, 2 * F3], bf16, tag="xlr")
                        nc.scalar.copy(xlr[:mm, :FE], psA[:mm, :FE])
                        nc.sync.dma_start(xlr_sh[li][n0:n0 + mm, :], xlr[:mm, :FE])

                HS = SHARD // 2
                HN = HS * NCORES
                nc.gpsimd.collective_compute(
                    "AllGather", mybir.AluOpType.bypass, replica_groups=RG,
                    ins=[xlr_sh[li][0:HS, :]], outs=[xlr_rows_cc[li][0:HN, :]])
                nc.gpsimd.collective_compute(
                    "AllGather", mybir.AluOpType.bypass, replica_groups=RG,
                    ins=[xlr_sh[li][HS:SHARD, :]], outs=[xlr_rows_cc[li][HN:N, :]])

                # gather source views: 4 direct-range classes (idx = src - r*32768)
                if not l3:
                    src_tabs = [xlr_rows_cc[li][r * 32768:, :] for r in range(m.NCLS)]
                    GELEM, GSTEP = FE, FE
                else:
                    src_tabs = [xlr_rows_cc[li][r * 32768:, :F3] for r in range(m.NCLS)]
                    GELEM, GSTEP = F3, FE

                # ---- phase B ----
                for s in range(m.NSB):
                    SG = m.sb_g[s]
                    go = m.sb_goff[s]
                    po = m.sb_poff[s]
                    NP = m.sb_p[s]
                    pairs = m.sb_pairs[s]
                    spairs = m.slot_pairs[s]
                    wo = go * P // 16
                    blocks = m.sb_blocks[s]
                    nblk = len(blocks)
                    xg = xg_pp[s % 2]
                    offs = m.sb_cls_off[s]
                    for r in range(m.NCLS):
                        nr = (offs[r + 1] - offs[r]) * P
                        nvalid = min(nr, ((int(m.gc_max[s, r]) + 127) // 128) * 128)
                        if nr == 0 or nvalid == 0:
                            continue
                        nc.gpsimd.dma_gather(
                            out_ap=xg[:, offs[r]:offs[r + 1], :GELEM], in_ap=src_tabs[r],
                            idxs_ap=srcw[:, wo + offs[r] * 8:wo + offs[r + 1] * 8],
                            num_idxs=nr, num_idxs_reg=nvalid, elem_size=GELEM,
                            elem_step=GSTEP, single_packet=False)

                    # local [xl|xr] rows per block: [P, SB, FE]
                    xrb = wpool.tile([P, SB, 2 * F3], bf16, tag="xrb")
                    if blocks[-1] * P + P > SHARD:
                        nc.vector.memset(xrb[:], 0.0)
                    for a, b in enumerate(blocks):
                        n0 = b * P
                        mm = min(P, SHARD - n0)
                        nc.sync.dma_start(xrb[:mm, a, :FE],
                                          xlr_sh[li][n0:n0 + mm, :])

                    # one-hot S [lane, dst-in-block] per PAIR (slot, block)
                    S = wpool.tile([P, m.NPmax, P], bf16, tag="S")
                    nc.vector.tensor_tensor(
                        out=S[:, :NP, :],
                        in0=iota[:, :NP * P].rearrange("p (g n) -> p g n", g=NP),
                        in1=dstrel[:, po:po + NP].to_broadcast([P, NP, P]),
                        op=OP.is_equal)

                    # z = xl[src] + xr[dst]; xr via PE: S_T = transpose(S),
                    # then psum_xr[slot] += S_T^T @ xr_blk over the slot's pairs
                    zl = wpool.tile([P, m.SBGmax, F3 if l3 else F1], bf16, tag="zl")
                    psxr_t = {}
                    nqp = (NP + 3) // 4
                    for q in range(nqp):
                        w4 = min(4, NP - 4 * q)
                        psST = ppST.tile([P, 4, P], bf16, tag="psST", space="PSUM")
                        for j in range(w4):
                            nc.tensor.transpose(psST[:, j, :], S[:, 4 * q + j, :], ident[:])
                        STt = wpool.tile([P, 4, P], bf16, tag="STt")
                        if q % 2 == 0:
                            nc.scalar.copy(STt[:, :w4, :], psST[:, :w4, :])
                        else:
                            nc.vector.tensor_copy(STt[:, :w4, :], psST[:, :w4, :])
                        for j in range(w4):
                            pid = 4 * q + j
                            sl, a = pairs[pid]
                            sq = sl // 4
                            if sq not in psxr_t:
                                t_xr = ppXR.tile([P, 4, F3], f32, tag="psXR", space="PSUM")
                                psxr_t[sq] = t_xr
                            plist = spairs[sl]
                            nc.tensor.matmul(psxr_t[sq][:, sl % 4, :Fl],
                                             lhsT=STt[:, j, :],
                                             rhs=xrb[:, a, Fl:FE],
                                             start=(pid == plist[0]),
                                             stop=(pid == plist[-1]))
                            if pid == NP - 1 or pairs[pid + 1][0] // 4 > sq:
                                w4s = min(4, SG - sq * 4)
                                nc.vector.tensor_tensor(
                                    out=zl[:, sq * 4:sq * 4 + w4s, :Fl],
                                    in0=xg[:, sq * 4:sq * 4 + w4s, :Fl],
                                    in1=psxr_t[sq][:, :w4s, :Fl], op=OP.add)
                                del psxr_t[sq]

                    # lrelu = max(0.2 z, z) on DVE, then *att, reduce -> score
                    nc.vector.scalar_tensor_tensor(
                        out=zl[:, :SG, :Fl], in0=zl[:, :SG, :Fl], scalar=NEG_SLOPE,
                        in1=zl[:, :SG, :Fl], op0=OP.mult, op1=OP.max)
                    nc.vector.tensor_tensor(
                        out=zl[:, :SG, :Fl], in0=zl[:, :SG, :Fl],
                        in1=arep_sb[li][:, :SG * Fl].rearrange("p (g f) -> p g f", g=SG),
                        op=OP.mult)
                    score = wpool.tile([P, m.SBGmax * H], f32, tag="score")
                    nc.vector.tensor_reduce(
                        out=score[:, :SG * H],
                        in_=zl[:, :SG, :Fl].rearrange("p g (h c) -> p g h c", h=H),
                        axis=X, op=OP.add)
                    wlhs = wpool.tile([P, m.SBGmax, F3 + H], bf16, tag="wlhs")
                    nc.scalar.activation(
                        wlhs[:, :SG, :Fl].rearrange("p g (h c) -> p g h c", h=H),
                        score[:, :SG * H].rearrange("p (g h) -> p g h", g=SG).to_broadcast([P, SG, H, Cl]),
                        AF.Exp)
                    nc.scalar.activation(
                        wlhs[:, :SG, Fl:Fl + H],
                        score[:, :SG * H].rearrange("p (g h) -> p g h", g=SG),
                        AF.Exp)
                    nc.vector.tensor_tensor(out=wlhs[:, :SG, :Fl],
                                            in0=xg[:, :SG, :Fl],
                                            in1=wlhs[:, :SG, :Fl], op=OP.mult)

                    # self-edge contribution from local rows (no gather):
                    # zs = xl[d]+xr[d]; wx = mult * [exp(a.lrelu(zs)) * xl | exp]
                    zs = tpool.tile([P, SB, F3], bf16, tag="zs")
                    nc.vector.tensor_tensor(out=zs[:, :nblk, :Fl],
                                            in0=xrb[:, :nblk, :Fl],
                                            in1=xrb[:, :nblk, Fl:FE], op=OP.add)
                    nc.vector.scalar_tensor_tensor(
                        out=zs[:, :nblk, :Fl], in0=zs[:, :nblk, :Fl],
                        scalar=NEG_SLOPE, in1=zs[:, :nblk, :Fl],
                        op0=OP.mult, op1=OP.max)
                    nc.vector.tensor_tensor(
                        out=zs[:, :nblk, :Fl], in0=zs[:, :nblk, :Fl],
                        in1=arep_sb[li][:, :nblk * Fl].rearrange("p (a f) -> p a f", a=nblk),
                        op=OP.mult)
                    ssc = tpool.tile([P, SB * H], f32, tag="ssc")
                    nc.vector.tensor_reduce(
                        out=ssc[:, :nblk * H],
                        in_=zs[:, :nblk, :Fl].rearrange("p a (h c) -> p a h c", h=H),
                        axis=X, op=OP.add)
                    wx = tpool.tile([P, SB, F3 + H], bf16, tag="wx")
                    nc.scalar.activation(
                        wx[:, :nblk, :Fl].rearrange("p a (h c) -> p a h c", h=H),
                        ssc[:, :nblk * H].rearrange("p (a h) -> p a h", a=nblk).to_broadcast([P, nblk, H, Cl]),
                        AF.Exp)
                    nc.scalar.activation(
                        wx[:, :nblk, Fl:Fl + H],
                        ssc[:, :nblk * H].rearrange("p (a h) -> p a h", a=nblk),
                        AF.Exp)
                    nc.vector.tensor_tensor(out=wx[:, :nblk, :Fl],
                                            in0=xrb[:, :nblk, :Fl],
                                            in1=wx[:, :nblk, :Fl], op=OP.mult)
                    b0 = blocks[0]
                    nc.vector.tensor_tensor(
                        out=wx[:, :nblk, :Fl + H], in0=wx[:, :nblk, :Fl + H],
                        in1=multb[:, b0:b0 + nblk, None].to_broadcast([P, nblk, Fl + H]),
                        op=OP.mult)

                    # aggregation: per block, accumulate S.T @ [w*xl | w]
                    bpairs = m.blk_pairs[s]
                    if not l3:
                        psB = ppB.tile([P, SB, F1 + H], f32, tag="psB", space="PSUM")
                        for a, b in enumerate(blocks):
                            plist = bpairs[a]
                            for i, pid in enumerate(plist):
                                nc.tensor.matmul(psB[:, a, :Fl + H], lhsT=S[:, pid, :],
                                                 rhs=wlhs[:, pairs[pid][0], :Fl + H],
                                                 start=(i == 0), stop=(i == len(plist) - 1))
                        nc.vector.tensor_tensor(
                            out=psB[:, :nblk, :Fl + H], in0=psB[:, :nblk, :Fl + H],
                            in1=wx[:, :nblk, :Fl + H], op=OP.add)
                        # superblock-batched tail
                        rec = tpool.tile([P, SB, H], f32, tag="rec")
                        nc.vector.reciprocal(rec[:, :nblk, :], psB[:, :nblk, Fl:Fl + H])
                        hb = tpool.tile([P, SB, F1], bf16, tag="hb")
                        nc.vector.tensor_tensor(
                            out=hb[:, :nblk, :].rearrange("p a (h c) -> p a h c", h=H),
                            in0=psB[:, :nblk, :Fl].rearrange("p a (h c) -> p a h c", h=H),
                            in1=rec[:, :nblk, :, None].to_broadcast([P, nblk, H, Cl]),
                            op=OP.mult)
                        nc.vector.tensor_tensor(
                            out=hb[:, :nblk, :],
                            in0=hb[:, :nblk, :],
                            in1=brep_sb[li][:, :nblk * F1].rearrange("p (a f) -> p a f", a=nblk),
                            op=OP.add)
                        rp = tpool.tile([P, SB, F1], bf16, tag="rp")
                        nc.scalar.activation(rp[:, :nblk, :], hb[:, :nblk, :], AF.Relu)
                        xm = tpool.tile([P, SB, F1], bf16, tag="xm")
                        nc.vector.tensor_scalar_min(out=xm[:, :nblk, :],
                                                    in0=hb[:, :nblk, :], scalar1=0.0)
                        ex = tpool.tile([P, SB, F1], f32, tag="ex")
                        nc.scalar.activation(ex[:, :nblk, :], xm[:, :nblk, :], AF.Exp)
                        ho = tpool.tile([P, SB, F1], bf16, tag="ho")
                        nc.vector.scalar_tensor_tensor(
                            out=ho[:, :nblk, :], in0=ex[:, :nblk, :], scalar=-1.0,
                            in1=rp[:, :nblk, :], op0=OP.add, op1=OP.add)
                        for a, b in enumerate(blocks):
                            n0 = b * P
                            mm = min(P, SHARD - n0)
                            psT = ppAT.tile([F1, P], bf16, tag="psAT", space="PSUM")
                            nc.tensor.transpose(psT[:, :mm], ho[:mm, a, :], ident[:mm, :mm])
                            hTn = tpool.tile([F1, P], bf16, tag="hTn")
                            nc.scalar.copy(hTn[:, :mm], psT[:, :mm])
                            # fused phase A of the next layer
                            FEn = 2 * (F1 if li == 0 else F3)
                            psA = ppAT.tile([P, 2 * F3], f32, tag="psAT", space="PSUM")
                            nc.tensor.matmul(psA[:mm, :FEn], lhsT=hTn[:, :mm],
                                             rhs=W_sb[li + 1][:], start=True, stop=True)
                            xlr = apool.tile([P, 2 * F3], bf16, tag="xlr")
                            nc.scalar.copy(xlr[:mm, :FEn], psA[:mm, :FEn])
                            nc.sync.dma_start(xlr_sh[li + 1][n0:n0 + mm, :], xlr[:mm, :FEn])
                    else:
                        for half in range((nblk + 1) // 2):
                            hb0 = 2 * half
                            hblocks = blocks[hb0:hb0 + 2]
                            nh = len(hblocks)
                            psB3 = ppB.tile([P, 2, F3 + H], f32, tag="psB", space="PSUM")
                            for a, b in enumerate(hblocks):
                                plist = bpairs[hb0 + a]
                                for i, pid in enumerate(plist):
                                    nc.tensor.matmul(psB3[:, a, :F3 + H], lhsT=S[:, pid, :],
                                                     rhs=wlhs[:, pairs[pid][0], :F3 + H],
                                                     start=(i == 0), stop=(i == len(plist) - 1))
                            nc.vector.tensor_tensor(
                                out=psB3[:, :nh, :F3 + H], in0=psB3[:, :nh, :F3 + H],
                                in1=wx[:, hb0:hb0 + nh, :F3 + H], op=OP.add)
                            rec = tpool.tile([P, 2, H], f32, tag="rec3")
                            nc.vector.reciprocal(rec[:, :nh, :], psB3[:, :nh, F3:F3 + H])
                            o3 = tpool.tile([P, 2, F3], f32, tag="o3")
                            nc.vector.tensor_tensor(
                                out=o3[:, :nh, :].rearrange("p a (h c) -> p a h c", h=H),
                                in0=psB3[:, :nh, :F3].rearrange("p a (h c) -> p a h c", h=H),
                                in1=rec[:, :nh, :, None].to_broadcast([P, nh, H, Cl]),
                                op=OP.mult)
                            m01 = tpool.tile([P, 2, C3], f32, tag="m01")
                            nc.vector.tensor_tensor(out=m01[:, :nh, :], in0=o3[:, :nh, 0:C3],
                                                    in1=o3[:, :nh, C3:2 * C3], op=OP.add)
                            m23 = tpool.tile([P, 2, C3], f32, tag="m23")
                            nc.vector.tensor_tensor(out=m23[:, :nh, :], in0=o3[:, :nh, 2 * C3:3 * C3],
                                                    in1=o3[:, :nh, 3 * C3:4 * C3], op=OP.add)
                            ms = tpool.tile([P, 2, C3], f32, tag="ms")
                            nc.vector.tensor_tensor(out=ms[:, :nh, :], in0=m01[:, :nh, :],
                                                    in1=m23[:, :nh, :], op=OP.add)
                            of = tpool.tile([P, 2, C3], f32, tag="of")
                            nc.vector.scalar_tensor_tensor(
                                out=of[:, :nh, :], in0=ms[:, :nh, :], scalar=0.25,
                                in1=brep_sb[2][:, :nh * C3].rearrange("p (a c) -> p a c", a=nh),
                                op0=OP.mult, op1=OP.add)
                            for a, b in enumerate(hblocks):
                                n0 = b * P
                                mm = min(P, SHARD - n0)
                                nc.sync.dma_start(out_d[n0:n0 + mm, :], of[:mm, a, :])

    nc.compile()
    return nc


def _prep_inputs(x, edge_index, Ws, atts):
    m = _preprocess(edge_index)
    ident = np.eye(P, dtype=np.float32).astype(BF16)
    iota = np.broadcast_to(np.arange(P, dtype=np.float32), (P, m.NPmax, P)) \
        .reshape(P, m.NPmax * P).astype(BF16).copy()
    common = {"ident": ident, "iota": iota}
    for li, ((Wl, Wr), a) in enumerate(zip(Ws, atts)):
        Fl = Wl.shape[1]
        common[f"W{li + 1}"] = np.concatenate([Wl, Wr], axis=1).astype(BF16)
        a_flat = np.asarray(a).reshape(Fl).astype(np.float32)
        common[f"arep{li + 1}"] = np.broadcast_to(a_flat, (P, m.SBGmax, Fl)) \
            .reshape(P, m.SBGmax * Fl).astype(BF16).copy()
    in_maps = []
    for c in range(NCORES):
        d = dict(common)
        d["xT"] = x[c * SHARD:(c + 1) * SHARD].T.astype(BF16).copy()
        d["srcw"] = m.src_w[c]
        d["dstrel"] = m.rel_pm[c]
        d["multb"] = m.multc[c]
        in_maps.append(d)
    return in_maps, m


def kernel(x, edge_index, W1l, W1r, a1, b1, W2l, W2r, a2, b2, W3l, W3r, a3, b3,
           _trace=False, _tmpdir=None):
    from concourse.bass_utils import run_bass_kernel_spmd

    x = np.asarray(x, dtype=np.float32)
    edge_index = np.asarray(edge_index, dtype=np.int32)
    in_maps, m = _prep_inputs(
        x, edge_index,
        [(np.asarray(W1l), np.asarray(W1r)), (np.asarray(W2l), np.asarray(W2r)),
         (np.asarray(W3l), np.asarray(W3r))],
        [a1, a2, a3])
    b1r = np.broadcast_to(np.asarray(b1, np.float32), (P, SB, F1)).reshape(P, SB * F1).astype(BF16).copy()
    b2r = np.broadcast_to(np.asarray(b2, np.float32), (P, SB, F1)).reshape(P, SB * F1).astype(BF16).copy()
    b3r = np.broadcast_to(np.asarray(b3, np.float32), (P, 2, C3)).reshape(P, 2 * C3).astype(np.float32).copy()
    for c in range(NCORES):
        in_maps[c]["brep1"] = b1r
        in_maps[c]["brep2"] = b2r
        in_maps[c]["brep3"] = b3r

    key = (m.Gtot, m.Ptot, tuple(m.sb_g), tuple(tuple(o) for o in m.sb_cls_off),
           tuple(tuple(pl) for pl in m.sb_pairs), tuple(map(tuple, m.gc_max)))
    if key not in _cache:
        _cache.clear()
        _cache[key] = _build_program(m)
    nc = _cache[key]

    res = run_bass_kernel_spmd(nc, in_maps, core_ids=list(range(NCORES)),
                               trace=_trace, tmpdir=_tmpdir)
    out = np.concatenate([res.results[c]["out_shard"] for c in range(NCORES)], axis=0)
    kernel._last_result = res
    return out
